# revision 13
# baseline (speedup 1.0000x reference)
"""Trainium2 Bass kernel for nn_EnhancedGNN (3-layer GCN + mean-pool + FC).

Contract: kernel(**inputs) takes FULL unsharded numpy inputs (keyed as in
setup_inputs) and returns the FULL [64, 1] float32 output. Internally the
work is sharded over 8 NeuronCores:

  - dst-sharded edge phases: core k owns 98 windows of 128 destination
    nodes. Edge source rows are gathered with dma_gather (int16 indices ->
    4 source chunks of 32768 rows) from a bf16 table with 256B rows,
    scaled in place by w*dinv_dst, and scatter-added via PE matmuls with
    one-hot(dst) selection matrices built on DVE.
  - gcn_norm folding: table rows are h*dinv_src; the per-edge scale is
    w_e*dinv_dst (host-baked); self-loops are dinv_dst*Town adds.
  - epilogues run on the Scalar engine (activation scale) and PE (bias
    via an appended ones-row and stacked [W; b] weights).
  - layer tables are exchanged with AllGather (bf16); mean-pool uses a
    host-built one-hot batch matmul + a tiny AllReduce.
"""

import math
import os
import sys
import types

import numpy as np

# ---------------------------------------------------------------- constants
N_NODES = 100000
F_IN = 16
N_GRAPHS = 64
P = 128
N_CORES = 8
W_PER_CORE = 98                      # windows of 128 dst nodes per core
NPC = W_PER_CORE * P                 # 12544 nodes per core
NODES_PAD = N_CORES * NPC            # 100352
CHUNK = 32768                        # src chunk (int16 index range)
N_CHUNKS = 4
GROUPS = [(0, 44), (44, 88), (88, 98)]   # window groups (small tail)
TD = 128                             # table row width (bf16 -> 256B rows)
MAX_CALL_BLOCKS = 72                 # max indices per dma_gather call

LAST_EXEC_TIME_NS = None
LAST_TRACE = None
LAST_RESULT = None


# ---------------------------------------------------------------- host prep
def _prep_edges(src, dst, w, dinv_n):
    E = src.shape[0]
    # window permutation: group 784 windows into 98 groups of 8 by similar
    # edge count; balances per-(chunk,window) slot padding across cores
    NWIN = NODES_PAD // P
    cntw = np.bincount(dst // P, minlength=NWIN)
    order = np.argsort(cntw, kind="stable")
    inv = np.zeros(NWIN, np.int64)
    inv[order] = (np.arange(NWIN) % N_CORES) * W_PER_CORE \
        + (np.arange(NWIN) // N_CORES)
    pos = inv[np.arange(N_NODES) // P] * P + (np.arange(N_NODES) % P)
    src = pos[src]
    dst = pos[dst]
    core = dst // NPC
    wl = (dst % NPC) // P            # local window 0..97
    ch = src // CHUNK                # source chunk 0..3

    key = (core * N_CHUNKS + ch) * W_PER_CORE + wl
    cnt = np.bincount(key, minlength=N_CORES * N_CHUNKS * W_PER_CORE)
    cnt = cnt.reshape(N_CORES, N_CHUNKS, W_PER_CORE)
    nblk = np.maximum(1, -(-cnt.max(axis=0) // P))   # [N_CHUNKS, W_PER_CORE]

    # skeleton: stream order (group, chunk, window, block)
    blocks = []            # (chunk, wloc, grp, start, stop)
    calls = []             # (grp, chunk, b0, b1)
    base_arr = np.zeros((N_CHUNKS, W_PER_CORE), np.int64)
    for g, (lo, hi) in enumerate(GROUPS):
        for c in range(N_CHUNKS):
            seg_b0 = len(blocks)
            for wloc in range(lo, hi):
                n = int(nblk[c, wloc])
                base_arr[c, wloc] = len(blocks) * P
                for j in range(n):
                    blocks.append((c, wloc, g, j == 0, j == n - 1))
            seg_b1 = len(blocks)
            for b0 in range(seg_b0, seg_b1, MAX_CALL_BLOCKS):
                calls.append((g, c, b0, min(b0 + MAX_CALL_BLOCKS, seg_b1)))
    NBLK = len(blocks)
    NSLOT = NBLK * P

    # per-edge slot position: base of its (chunk, window) run + rank inside
    order = np.lexsort((wl, ch, core))
    skey = key[order]
    starts = np.flatnonzero(np.r_[True, skey[1:] != skey[:-1]])
    sizes = np.diff(np.r_[starts, E])
    rank = np.arange(E, dtype=np.int64) - np.repeat(starts, sizes)
    pos_sorted = base_arr[ch[order], wl[order]] + rank
    core_sorted = core[order]

    # dinv by *position*: dinv_n is per original node id
    dinv_pos = np.zeros(NODES_PAD, np.float64)
    dinv_pos[pos] = dinv_n

    idx16 = np.zeros((N_CORES, NSLOT), np.int16)
    dstrel = np.zeros((N_CORES, NSLOT), np.float32)
    wdi = np.zeros((N_CORES, NSLOT), np.float32)
    src_s = src[order]
    dst_s = dst[order]
    w_s = w[order].astype(np.float64) * dinv_pos[dst_s]
    ch_s = ch[order]
    wl_s = wl[order]
    for k in range(N_CORES):
        m = core_sorted == k
        p = pos_sorted[m]
        idx16[k, p] = (src_s[m] - ch_s[m] * CHUNK).astype(np.int16)
        dstrel[k, p] = (dst_s[m] - (k * NPC + wl_s[m] * P)).astype(np.float32)
        wdi[k, p] = w_s[m].astype(np.float32)
    # padding slots: dstrel stays 0 but wdi=0 makes their contribution zero

    # idx wrap: idx i -> [i % 16, i // 16], replicated over 8 partition groups
    idxw = np.tile(
        idx16.reshape(N_CORES, NSLOT // 16, 16).transpose(0, 2, 1), (1, 8, 1)
    )                                                   # [8cores,128,NSLOT/16]
    import ml_dtypes
    dstrel_st = np.ascontiguousarray(
        dstrel.reshape(N_CORES, NBLK, P).transpose(0, 2, 1)
        .astype(ml_dtypes.bfloat16))
    wdi_st = np.ascontiguousarray(
        wdi.reshape(N_CORES, NBLK, P).transpose(0, 2, 1)
        .astype(ml_dtypes.bfloat16))

    meta = {"nblk": nblk, "blocks": blocks, "calls": calls,
            "NBLK": NBLK, "NSLOT": NSLOT}
    return meta, idxw, dstrel_st, wdi_st, pos


def _prep_nodes(x, batch, pos, dinv_n):
    import ml_dtypes
    xd = (x.astype(np.float64) * dinv_n[:, None].astype(np.float64))
    # Town layout: [P, W, TD] bf16, cols 0:F_IN = x*dinv
    town = np.zeros((NODES_PAD, TD), np.float32)
    town[pos, :F_IN] = xd.astype(np.float32)
    town_r = (
        town.reshape(N_CORES, W_PER_CORE, P, TD)
        .transpose(0, 2, 1, 3)
        .reshape(N_CORES, P, W_PER_CORE * TD)
        .astype(ml_dtypes.bfloat16)
    )
    town_r = np.ascontiguousarray(town_r)
    # pooling one-hot S: [P, W, 64] bf16
    bf = np.full((NODES_PAD,), -1, np.int64)
    bf[pos] = batch.astype(np.int64)
    bfr = bf.reshape(N_CORES, W_PER_CORE, P).transpose(0, 2, 1)
    S = (bfr[:, :, :, None] == np.arange(N_GRAPHS)[None, None, None, :])
    Sbf = np.ascontiguousarray(
        S.reshape(N_CORES, P, W_PER_CORE * N_GRAPHS)
        .astype(ml_dtypes.bfloat16))
    dinvp = np.zeros(NODES_PAD, np.float32)
    dinvp[pos] = dinv_n
    dinvf = np.ascontiguousarray(
        dinvp.reshape(N_CORES, W_PER_CORE, P).transpose(0, 2, 1))
    # T1 table: [NODES_PAD, TD] bf16, cols 0:F_IN = x*dinv
    T1bf = np.zeros((NODES_PAD, TD), ml_dtypes.bfloat16)
    T1bf[pos, :F_IN] = xd.astype(ml_dtypes.bfloat16)
    return town_r, Sbf, dinvf, T1bf


# ------------------------------------------------------------- bass builder
def _build_nc(meta):
    import concourse.bacc as bacc
    import concourse.mybir as mybir
    import concourse.tile as tile
    from concourse.masks import make_identity

    f32 = mybir.dt.float32
    bfdt = mybir.dt.bfloat16
    i16 = mybir.dt.int16
    i32 = mybir.dt.int32
    AF = mybir.ActivationFunctionType
    OP = mybir.AluOpType

    NBLK = meta["NBLK"]
    NSLOT = meta["NSLOT"]
    blocks = meta["blocks"]
    calls = meta["calls"]

    nc = bacc.Bacc("TRN2", target_bir_lowering=False, debug=False,
                   num_devices=N_CORES, num_swdge_queues=2,
                   dynamic_dma_scratch_size=32768)

    # ------------------------------------------------- I/O declarations
    T1_t = nc.dram_tensor("T1bf", [NODES_PAD, TD], bfdt,
                          kind="ExternalInput")
    town_t = nc.dram_tensor("town", [P, W_PER_CORE * TD], bfdt,
                            kind="ExternalInput")
    dinv_t = nc.dram_tensor("dinvr", [P, W_PER_CORE], f32,
                            kind="ExternalInput")
    idx_t = nc.dram_tensor("idxw", [P, NSLOT // 16], i16, kind="ExternalInput")
    dst_t = nc.dram_tensor("dstrel", [P, NBLK], bfdt, kind="ExternalInput")
    wdi_t = nc.dram_tensor("wdi", [P, NBLK], bfdt, kind="ExternalInput")
    S_t = nc.dram_tensor("Sbf", [P, W_PER_CORE * N_GRAPHS], bfdt,
                         kind="ExternalInput")
    W1s_t = nc.dram_tensor("W1s", [F_IN + 1, 64], f32, kind="ExternalInput")
    W2s_t = nc.dram_tensor("W2s", [65, 128], f32, kind="ExternalInput")
    W3_t = nc.dram_tensor("W3", [128, 64], f32, kind="ExternalInput")
    Wfc_t = nc.dram_tensor("Wfc", [64, 1], f32, kind="ExternalInput")
    b3_t = nc.dram_tensor("b3r", [P, 64], f32, kind="ExternalInput")
    bfc_t = nc.dram_tensor("bfcr", [64, 1], f32, kind="ExternalInput")
    out_t = nc.dram_tensor("out", [64, 1], f32, kind="ExternalOutput")

    RG = [list(range(N_CORES))]

    with tile.TileContext(nc) as tc:
        with (
            tc.tile_pool(name="dram", bufs=1, space="DRAM") as dram,
            tc.tile_pool(name="const", bufs=1) as const,
            tc.tile_pool(name="cmat", bufs=2) as cpool,
            tc.tile_pool(name="gat", bufs=2) as gpool,
            tc.tile_pool(name="epi", bufs=2) as epool,
            tc.tile_pool(name="sps", bufs=1, space="PSUM") as spool,
        ):
            # DRAM buffers
            T2 = dram.tile([NODES_PAD, TD], bfdt, addr_space="Shared")
            T3 = dram.tile([NODES_PAD, TD], bfdt, addr_space="Shared")
            ag2 = dram.tile([NPC, TD], bfdt)
            ag3 = dram.tile([NPC, TD], bfdt)
            poolin = dram.tile([64, 65], f32)
            poolred = dram.tile([64, 65], f32, addr_space="Shared")

            # constants / resident streams
            iota_i = const.tile([P, P], i32)
            nc.gpsimd.iota(iota_i[:], pattern=[[1, P]], channel_multiplier=0)
            iota_bf = const.tile([P, P], bfdt)
            nc.vector.tensor_copy(out=iota_bf[:], in_=iota_i[:])
            ident = const.tile([P, P], f32)
            make_identity(nc, ident[:])

            sdstb = const.tile([P, NBLK], bfdt)
            nc.sync.dma_start(out=sdstb[:], in_=dst_t[:])
            swdi = const.tile([P, NBLK], bfdt)
            nc.sync.dma_start(out=swdi[:], in_=wdi_t[:])
            sS = const.tile([P, W_PER_CORE * N_GRAPHS], bfdt)
            nc.sync.dma_start(out=sS[:], in_=S_t[:])
            sW1 = const.tile([F_IN + 1, 64], f32)
            nc.sync.dma_start(out=sW1[:], in_=W1s_t[:])
            sW2 = const.tile([65, 128], f32)
            nc.sync.dma_start(out=sW2[:], in_=W2s_t[:])
            sW3 = const.tile([128, 64], f32)
            nc.sync.dma_start(out=sW3[:], in_=W3_t[:])
            sWfc = const.tile([64, 1], f32)
            nc.sync.dma_start(out=sWfc[:], in_=Wfc_t[:])
            sb3 = const.tile([P, 64], f32)
            nc.sync.dma_start(out=sb3[:], in_=b3_t[:])
            sbfc = const.tile([64, 1], f32)
            nc.sync.dma_start(out=sbfc[:], in_=bfc_t[:])

            Town = const.tile([P, W_PER_CORE * TD], bfdt)
            nc.sync.dma_start(out=Town[:], in_=town_t[:])
            dinv = const.tile([P, W_PER_CORE], f32)
            nc.sync.dma_start(out=dinv[:], in_=dinv_t[:])

            # z accumulator in SBUF (f32)
            z_sb = const.tile([P, W_PER_CORE * 64], f32)

            # --------------------------------------------- shared helpers
            def edge_phase(layer, Ttab, F_rhs, epilogue, post_group=None):
                acc = None
                qsel = 0
                for g, (lo, hi) in enumerate(GROUPS):
                    for (cg, cc, b0, b1) in calls:
                        if cg != g:
                            continue
                        qsel ^= 1
                        nb = b1 - b0
                        n = nb * P
                        sid = gpool.tile([P, MAX_CALL_BLOCKS * 8], i16,
                                         tag="sid", bufs=3)
                        nc.sync.dma_start(out=sid[:, :nb * 8],
                                          in_=idx_t[:, b0 * 8:b1 * 8])
                        gt = gpool.tile([P, MAX_CALL_BLOCKS, TD], bfdt,
                                        tag="g", bufs=2)
                        c0 = cc * CHUNK
                        c1 = min((cc + 1) * CHUNK, NODES_PAD)
                        nc.gpsimd.dma_gather(
                            out_ap=gt[:, :nb, :],
                            in_ap=Ttab[c0:c1, :],
                            idxs_ap=sid[:, :nb * 8],
                            num_idxs=n, num_idxs_reg=n, elem_size=TD,
                            single_packet=False, queue_num=qsel,
                        )
                        Cb = cpool.tile([P, MAX_CALL_BLOCKS, P], bfdt,
                                        tag="C", bufs=2)
                        nc.vector.tensor_tensor(
                            out=Cb[:, :nb, :],
                            in0=iota_bf[:].unsqueeze(1)
                                .to_broadcast([P, nb, P]),
                            in1=sdstb[:, b0:b1].unsqueeze(2)
                                .to_broadcast([P, nb, P]),
                            op=OP.is_equal,
                        )
                        gts = gpool.tile([P, MAX_CALL_BLOCKS, 64], bfdt,
                                         tag="gs", bufs=2)
                        nc.vector.tensor_tensor(
                            out=gts[:, :nb, 0:F_rhs],
                            in0=gt[:, :nb, 0:F_rhs],
                            in1=swdi[:, b0:b1].unsqueeze(2)
                                .to_broadcast([P, nb, F_rhs]),
                            op=OP.mult,
                        )
                        for j in range(nb):
                            b = b0 + j
                            c, wloc, _, st, sp = blocks[b]
                            if st:
                                acc = spool.tile([P, 64], f32, tag="acc",
                                                 bufs=4, name="zacc")
                            nc.tensor.matmul(
                                out=acc[:, 0:F_rhs],
                                lhsT=Cb[:, j, :], rhs=gts[:, j, 0:F_rhs],
                                start=st, stop=sp, skip_group_check=True,
                            )
                            if sp:
                                zs = z_sb[:, wloc * 64:wloc * 64 + F_rhs]
                                if c == 0:
                                    nc.vector.tensor_copy(
                                        out=zs, in_=acc[:, 0:F_rhs])
                                else:
                                    nc.vector.tensor_tensor(
                                        out=zs, in0=zs, in1=acc[:, 0:F_rhs],
                                        op=OP.add)
                    for wloc in range(lo, hi):
                        epilogue(wloc,
                                 z_sb[:, wloc * 64:wloc * 64 + F_rhs])
                    if post_group is not None:
                        post_group(lo, hi)

            # --------------------------------------------- layer 1
            def epi1(w, zsl):
                # self loop: e1 = z + dinv*Town  (z already has w*dinv_dst)
                selfp = epool.tile([P, F_IN], f32, tag="selfp")
                nc.scalar.activation(
                    out=selfp[:], in_=Town[:, w * TD:w * TD + F_IN],
                    func=AF.Copy, scale=dinv[:, w:w + 1])
                e1 = epool.tile([P, F_IN + 1], f32, tag="e1")
                nc.vector.tensor_tensor(out=e1[:, 0:F_IN], in0=zsl,
                                        in1=selfp[:], op=OP.add)
                nc.vector.memset(e1[:, F_IN:F_IN + 1], 1.0)
                tp = spool.tile([P, P], f32, tag="sc1")
                nc.tensor.transpose(out=tp[:F_IN + 1, :], in_=e1[:],
                                    identity=ident[:])
                zT = epool.tile([F_IN + 1, P], f32, tag="zT1")
                nc.vector.tensor_copy(out=zT[:], in_=tp[:F_IN + 1, :])
                hp = spool.tile([P, P], f32, tag="sc2")
                nc.tensor.matmul(out=hp[:, 0:64], lhsT=zT[:], rhs=sW1[:],
                                 start=True, stop=True, skip_group_check=True)
                # Town <- relu(hp) * dinv   (bf16)
                nc.scalar.activation(
                    out=Town[:, w * TD:w * TD + 64], in_=hp[:, 0:64],
                    func=AF.Relu, scale=dinv[:, w:w + 1],
                )
                nc.sync.dma_start(out=ag2[w * P:(w + 1) * P, :],
                                  in_=Town[:, w * TD:(w + 1) * TD])

            edge_phase(1, T1_t, F_IN, epi1)
            nc.gpsimd.collective_compute(
                "AllGather", OP.bypass, replica_groups=RG,
                ins=[ag2.opt()], outs=[T2.opt()],
            )

            # --------------------------------------------- layer 2
            def epi2(w, zsl):
                selfp = epool.tile([P, 64], f32, tag="selfp2")
                nc.scalar.activation(
                    out=selfp[:], in_=Town[:, w * TD:w * TD + 64],
                    func=AF.Copy, scale=dinv[:, w:w + 1])
                e1 = epool.tile([P, 65], f32, tag="e1f2")
                nc.vector.tensor_tensor(out=e1[:, 0:64], in0=zsl,
                                        in1=selfp[:], op=OP.add)
                nc.vector.memset(e1[:, 64:65], 1.0)
                tp = spool.tile([P, P], f32, tag="sc1")
                nc.tensor.transpose(out=tp[:65, :], in_=e1[:],
                                    identity=ident[:])
                zT = epool.tile([65, P], f32, tag="zT2")
                nc.vector.tensor_copy(out=zT[:], in_=tp[:65, :])
                hp = spool.tile([P, P], f32, tag="sc2")
                nc.tensor.matmul(out=hp[:], lhsT=zT[:], rhs=sW2[:],
                                 start=True, stop=True, skip_group_check=True)
                h2 = epool.tile([P, P], f32, tag="h2r")
                nc.scalar.activation(out=h2[:], in_=hp[:], func=AF.Relu)
                tp2 = spool.tile([P, P], f32, tag="sc1")
                nc.tensor.transpose(out=tp2[:], in_=h2[:], identity=ident[:])
                h2T = epool.tile([P, P], f32, tag="h2T")
                nc.vector.tensor_copy(out=h2T[:], in_=tp2[:])
                mp = spool.tile([P, P], f32, tag="sc2")
                nc.tensor.matmul(out=mp[:, 0:64], lhsT=h2T[:], rhs=sW3[:],
                                 start=True, stop=True, skip_group_check=True)
                nc.scalar.activation(
                    out=Town[:, w * TD:w * TD + 64], in_=mp[:, 0:64],
                    func=AF.Copy, scale=dinv[:, w:w + 1],
                )
                nc.sync.dma_start(out=ag3[w * P:(w + 1) * P, :],
                                  in_=Town[:, w * TD:(w + 1) * TD])

            edge_phase(2, T2, 64, epi2)
            nc.gpsimd.collective_compute(
                "AllGather", OP.bypass, replica_groups=RG,
                ins=[ag3.opt()], outs=[T3.opt()],
            )

            # --------------------------------------------- layer 3 + pool
            pool_ps = spool.tile([P, 512], f32, tag="sc3")

            def epi3(w, zsl):
                selfp = epool.tile([P, 64], f32, tag="selfp2")
                nc.scalar.activation(
                    out=selfp[:], in_=Town[:, w * TD:w * TD + 64],
                    func=AF.Copy, scale=dinv[:, w:w + 1])
                e1 = epool.tile([P, 64], f32, tag="e1f")
                nc.vector.tensor_tensor(out=e1[:], in0=zsl, in1=selfp[:],
                                        op=OP.add)
                e2 = epool.tile([P, 64], f32, tag="e2f")
                nc.vector.tensor_tensor(out=e2[:], in0=e1[:], in1=sb3[:],
                                        op=OP.add)
                h3e = epool.tile([P, 65], bfdt, tag="h3e")
                nc.scalar.activation(out=h3e[:, 0:64], in_=e2[:],
                                     func=AF.Relu)
                nc.vector.memset(h3e[:, 64:65], 1.0)
                nc.tensor.matmul(
                    out=pool_ps[:64, 0:65],
                    lhsT=sS[:, w * N_GRAPHS:(w + 1) * N_GRAPHS],
                    rhs=h3e[:],
                    start=(w == 0), stop=(w == W_PER_CORE - 1),
                    skip_group_check=True,
                )

            edge_phase(3, T3, 64, epi3)

            poolsb = epool.tile([64, 65], f32, tag="poolsb")
            nc.vector.tensor_copy(out=poolsb[:], in_=pool_ps[:64, 0:65])
            nc.sync.dma_start(out=poolin[:], in_=poolsb[:])
            nc.gpsimd.collective_compute(
                "AllReduce", OP.add, replica_groups=RG,
                ins=[poolin.opt()], outs=[poolred.opt()],
            )
            pr = epool.tile([64, 65], f32, tag="pr")
            nc.sync.dma_start(out=pr[:], in_=poolred[:])
            cntc = epool.tile([64, 1], f32, tag="cntc")
            nc.vector.tensor_scalar(out=cntc[:], in0=pr[:, 64:65],
                                    scalar1=1.0, scalar2=None, op0=OP.max)
            rcnt = epool.tile([64, 1], f32, tag="rcnt")
            nc.vector.reciprocal(out=rcnt[:], in_=cntc[:])
            mean = epool.tile([64, 64], f32, tag="mean")
            nc.vector.tensor_scalar(out=mean[:], in0=pr[:, 0:64],
                                    scalar1=rcnt[:], scalar2=None,
                                    op0=OP.mult)
            tpf = spool.tile([P, P], f32, tag="sc1")
            nc.tensor.transpose(out=tpf[:64, :64], in_=mean[:],
                                identity=ident[:64, :64])
            meanT = epool.tile([64, 64], f32, tag="meanT")
            nc.vector.tensor_copy(out=meanT[:], in_=tpf[:64, :64])
            op_ps = spool.tile([P, P], f32, tag="sc2")
            nc.tensor.matmul(out=op_ps[:64, 0:1], lhsT=meanT[:], rhs=sWfc[:],
                             start=True, stop=True, skip_group_check=True)
            ob = epool.tile([64, 1], f32, tag="ob")
            nc.vector.tensor_tensor(out=ob[:], in0=op_ps[:64, 0:1],
                                    in1=sbfc[:], op=OP.add)
            nc.sync.dma_start(out=out_t[:], in_=ob[:])

    nc.finalize()
    return nc


# ------------------------------------------------------------------ runner
def _install_ntff_shim():
    try:
        import antenv
        if hasattr(antenv, "axon_hooks"):
            return
        mod = types.ModuleType("antenv.axon_hooks")
        mod._hook = None
        mod.set_axon_ntff_profile_hook = lambda h: setattr(mod, "_hook", h)
        mod.get_axon_ntff_profile_hook = lambda: mod._hook
        sys.modules["antenv.axon_hooks"] = mod
        antenv.axon_hooks = mod
        from trn_agent_boot.trn_boot import _ntff_profile_via_ctypes
        mod._hook = _ntff_profile_via_ctypes("/opt/axon/libaxon_pjrt.so")
    except Exception:
        pass


def kernel(x, edge_index, edge_weight, batch, W1, b1, W2, b2, W3, b3,
           Wfc, bfc):
    global LAST_EXEC_TIME_NS, LAST_TRACE, LAST_RESULT

    x = np.asarray(x, dtype=np.float32)
    ei = np.asarray(edge_index)
    src = ei[0].astype(np.int64)
    dst = ei[1].astype(np.int64)
    w = np.asarray(edge_weight, dtype=np.float32)
    batch = np.asarray(batch)

    deg = np.bincount(dst, weights=w.astype(np.float64),
                      minlength=N_NODES) + 1.0
    dinv_n = (1.0 / np.sqrt(deg)).astype(np.float32)
    meta, idxw, dstrel_st, wdi_st, pos = _prep_edges(src, dst, w, dinv_n)
    town_r, Sbf, dinvf, T1bf = _prep_nodes(x, batch, pos, dinv_n)

    W1 = np.asarray(W1, np.float32)
    W2 = np.asarray(W2, np.float32)
    W3 = np.asarray(W3, np.float32)
    Wfc = np.asarray(Wfc, np.float32).reshape(64, 1)
    W1s = np.vstack([W1, np.asarray(b1, np.float32).reshape(1, 64)])
    W2s = np.vstack([W2, np.asarray(b2, np.float32).reshape(1, 128)])
    b3r = np.tile(np.asarray(b3, np.float32).reshape(1, 64), (P, 1))
    bfcr = np.tile(np.asarray(bfc, np.float32).reshape(1, 1), (64, 1))

    nc = _build_nc(meta)

    in_maps = []
    for k in range(N_CORES):
        in_maps.append({
            "town": town_r[k], "idxw": idxw[k], "dstrel": dstrel_st[k],
            "wdi": wdi_st[k], "Sbf": Sbf[k], "T1bf": T1bf,
            "dinvr": dinvf[k],
            "W1s": W1s, "W2s": W2s, "W3": W3, "Wfc": Wfc,
            "b3r": b3r, "bfcr": bfcr,
        })

    trace = os.environ.get("BASS_GNN_TRACE", "") == "1"
    if trace:
        _install_ntff_shim()
        from concourse import bass_utils as _bu
        _bu.upload_artifacts = lambda tmpdir: tmpdir

    from concourse.bass_utils import run_bass_kernel_spmd
    res = run_bass_kernel_spmd(
        nc, in_maps, core_ids=list(range(N_CORES)), trace=trace,
    )
    LAST_RESULT = res
    if trace:
        LAST_EXEC_TIME_NS = res.exec_time_ns
        LAST_TRACE = (res.instructions_and_trace[1]
                      if res.instructions_and_trace else None)
    return np.asarray(res.results[0]["out"], dtype=np.float32)


# revision 14
# speedup vs baseline: 1.0177x; 1.0177x over previous
"""Trainium2 Bass kernel for nn_EnhancedGNN (3-layer GCN + mean-pool + FC).

Contract: kernel(**inputs) takes FULL unsharded numpy inputs (keyed as in
setup_inputs) and returns the FULL [64, 1] float32 output. Internally the
work is sharded over 8 NeuronCores:

  - dst-sharded edge phases: core k owns 98 windows of 128 destination
    nodes. Edge source rows are gathered with dma_gather (int16 indices ->
    4 source chunks of 32768 rows) from a bf16 table with 256B rows,
    scaled in place by w*dinv_dst, and scatter-added via PE matmuls with
    one-hot(dst) selection matrices built on DVE.
  - gcn_norm folding: table rows are h*dinv_src; the per-edge scale is
    w_e*dinv_dst (host-baked); self-loops are dinv_dst*Town adds.
  - epilogues run on the Scalar engine (activation scale) and PE (bias
    via an appended ones-row and stacked [W; b] weights).
  - layer tables are exchanged with AllGather (bf16); mean-pool uses a
    host-built one-hot batch matmul + a tiny AllReduce.
"""

import math
import os
import sys
import types

import numpy as np

# ---------------------------------------------------------------- constants
N_NODES = 100000
F_IN = 16
N_GRAPHS = 64
P = 128
N_CORES = 8
W_PER_CORE = 98                      # windows of 128 dst nodes per core
NPC = W_PER_CORE * P                 # 12544 nodes per core
NODES_PAD = N_CORES * NPC            # 100352
CHUNK = 32768                        # src chunk (int16 index range)
N_CHUNKS = 4
GROUPS = [(0, 44), (44, 88), (88, 98)]   # window groups (small tail)
TD = 128                             # table row width (bf16 -> 256B rows)
MAX_CALL_BLOCKS = 72                 # max indices per dma_gather call

LAST_EXEC_TIME_NS = None
LAST_TRACE = None
LAST_RESULT = None


# ---------------------------------------------------------------- host prep
def _prep_edges(src, dst, w, dinv_n):
    E = src.shape[0]
    # window permutation: group 784 windows into 98 groups of 8 by similar
    # edge count; balances per-(chunk,window) slot padding across cores
    NWIN = NODES_PAD // P
    cntw = np.bincount(dst // P, minlength=NWIN)
    order = np.argsort(cntw, kind="stable")
    inv = np.zeros(NWIN, np.int64)
    inv[order] = (np.arange(NWIN) % N_CORES) * W_PER_CORE \
        + (np.arange(NWIN) // N_CORES)
    pos = inv[np.arange(N_NODES) // P] * P + (np.arange(N_NODES) % P)
    src = pos[src]
    dst = pos[dst]
    core = dst // NPC
    wl = (dst % NPC) // P            # local window 0..97
    ch = src // CHUNK                # source chunk 0..3

    key = (core * N_CHUNKS + ch) * W_PER_CORE + wl
    cnt = np.bincount(key, minlength=N_CORES * N_CHUNKS * W_PER_CORE)
    cnt = cnt.reshape(N_CORES, N_CHUNKS, W_PER_CORE)
    nblk = np.maximum(1, -(-cnt.max(axis=0) // P))   # [N_CHUNKS, W_PER_CORE]

    # skeleton: stream order (group, chunk, window, block)
    blocks = []            # (chunk, wloc, grp, start, stop)
    calls = []             # (grp, chunk, b0, b1)
    base_arr = np.zeros((N_CHUNKS, W_PER_CORE), np.int64)
    for g, (lo, hi) in enumerate(GROUPS):
        for c in range(N_CHUNKS):
            seg_b0 = len(blocks)
            for wloc in range(lo, hi):
                n = int(nblk[c, wloc])
                base_arr[c, wloc] = len(blocks) * P
                for j in range(n):
                    blocks.append((c, wloc, g, j == 0, j == n - 1))
            seg_b1 = len(blocks)
            for b0 in range(seg_b0, seg_b1, MAX_CALL_BLOCKS):
                calls.append((g, c, b0, min(b0 + MAX_CALL_BLOCKS, seg_b1)))
    NBLK = len(blocks)
    NSLOT = NBLK * P

    # per-edge slot position: base of its (chunk, window) run + rank inside
    order = np.lexsort((wl, ch, core))
    skey = key[order]
    starts = np.flatnonzero(np.r_[True, skey[1:] != skey[:-1]])
    sizes = np.diff(np.r_[starts, E])
    rank = np.arange(E, dtype=np.int64) - np.repeat(starts, sizes)
    pos_sorted = base_arr[ch[order], wl[order]] + rank
    core_sorted = core[order]

    # dinv by *position*: dinv_n is per original node id
    dinv_pos = np.zeros(NODES_PAD, np.float64)
    dinv_pos[pos] = dinv_n

    idx16 = np.zeros((N_CORES, NSLOT), np.int16)
    dstrel = np.zeros((N_CORES, NSLOT), np.float32)
    wdi = np.zeros((N_CORES, NSLOT), np.float32)
    src_s = src[order]
    dst_s = dst[order]
    w_s = w[order].astype(np.float64) * dinv_pos[dst_s]
    ch_s = ch[order]
    wl_s = wl[order]
    for k in range(N_CORES):
        m = core_sorted == k
        p = pos_sorted[m]
        idx16[k, p] = (src_s[m] - ch_s[m] * CHUNK).astype(np.int16)
        dstrel[k, p] = (dst_s[m] - (k * NPC + wl_s[m] * P)).astype(np.float32)
        wdi[k, p] = w_s[m].astype(np.float32)
    # padding slots: dstrel stays 0 but wdi=0 makes their contribution zero

    # idx wrap: idx i -> [i % 16, i // 16], replicated over 8 partition groups
    idxw = np.tile(
        idx16.reshape(N_CORES, NSLOT // 16, 16).transpose(0, 2, 1), (1, 8, 1)
    )                                                   # [8cores,128,NSLOT/16]
    import ml_dtypes
    dstrel_st = np.ascontiguousarray(
        dstrel.reshape(N_CORES, NBLK, P).transpose(0, 2, 1)
        .astype(ml_dtypes.bfloat16))
    wdi_st = np.ascontiguousarray(
        wdi.reshape(N_CORES, NBLK, P).transpose(0, 2, 1)
        .astype(ml_dtypes.bfloat16))

    meta = {"nblk": nblk, "blocks": blocks, "calls": calls,
            "NBLK": NBLK, "NSLOT": NSLOT}
    return meta, idxw, dstrel_st, wdi_st, pos


def _prep_nodes(x, batch, pos, dinv_n):
    import ml_dtypes
    xd = (x.astype(np.float64) * dinv_n[:, None].astype(np.float64))
    # Town layout: [P, W, TD] bf16, cols 0:F_IN = x*dinv
    town = np.zeros((NODES_PAD, TD), np.float32)
    town[pos, :F_IN] = xd.astype(np.float32)
    town_r = (
        town.reshape(N_CORES, W_PER_CORE, P, TD)
        .transpose(0, 2, 1, 3)
        .reshape(N_CORES, P, W_PER_CORE * TD)
        .astype(ml_dtypes.bfloat16)
    )
    town_r = np.ascontiguousarray(town_r)
    # pooling one-hot S: [P, W, 64] bf16
    bf = np.full((NODES_PAD,), -1, np.int64)
    bf[pos] = batch.astype(np.int64)
    bfr = bf.reshape(N_CORES, W_PER_CORE, P).transpose(0, 2, 1)
    S = (bfr[:, :, :, None] == np.arange(N_GRAPHS)[None, None, None, :])
    Sbf = np.ascontiguousarray(
        S.reshape(N_CORES, P, W_PER_CORE * N_GRAPHS)
        .astype(ml_dtypes.bfloat16))
    dinvp = np.zeros(NODES_PAD, np.float32)
    dinvp[pos] = dinv_n
    dinvf = np.ascontiguousarray(
        dinvp.reshape(N_CORES, W_PER_CORE, P).transpose(0, 2, 1))
    # T1 table: [NODES_PAD, TD] bf16, cols 0:F_IN = x*dinv
    T1bf = np.zeros((NODES_PAD, TD), ml_dtypes.bfloat16)
    T1bf[pos, :F_IN] = xd.astype(ml_dtypes.bfloat16)
    return town_r, Sbf, dinvf, T1bf


# ------------------------------------------------------------- bass builder
def _build_nc(meta):
    import concourse.bacc as bacc
    import concourse.mybir as mybir
    import concourse.tile as tile
    from concourse.masks import make_identity

    f32 = mybir.dt.float32
    bfdt = mybir.dt.bfloat16
    i16 = mybir.dt.int16
    i32 = mybir.dt.int32
    AF = mybir.ActivationFunctionType
    OP = mybir.AluOpType

    NBLK = meta["NBLK"]
    NSLOT = meta["NSLOT"]
    blocks = meta["blocks"]
    calls = meta["calls"]

    nc = bacc.Bacc("TRN2", target_bir_lowering=False, debug=False,
                   num_devices=N_CORES, num_swdge_queues=4)

    # ------------------------------------------------- I/O declarations
    T1_t = nc.dram_tensor("T1bf", [NODES_PAD, TD], bfdt,
                          kind="ExternalInput")
    town_t = nc.dram_tensor("town", [P, W_PER_CORE * TD], bfdt,
                            kind="ExternalInput")
    dinv_t = nc.dram_tensor("dinvr", [P, W_PER_CORE], f32,
                            kind="ExternalInput")
    idx_t = nc.dram_tensor("idxw", [P, NSLOT // 16], i16, kind="ExternalInput")
    dst_t = nc.dram_tensor("dstrel", [P, NBLK], bfdt, kind="ExternalInput")
    wdi_t = nc.dram_tensor("wdi", [P, NBLK], bfdt, kind="ExternalInput")
    S_t = nc.dram_tensor("Sbf", [P, W_PER_CORE * N_GRAPHS], bfdt,
                         kind="ExternalInput")
    W1s_t = nc.dram_tensor("W1s", [F_IN + 1, 64], f32, kind="ExternalInput")
    W2s_t = nc.dram_tensor("W2s", [65, 128], f32, kind="ExternalInput")
    W3_t = nc.dram_tensor("W3", [128, 64], f32, kind="ExternalInput")
    Wfc_t = nc.dram_tensor("Wfc", [64, 1], f32, kind="ExternalInput")
    b3_t = nc.dram_tensor("b3r", [P, 64], f32, kind="ExternalInput")
    bfc_t = nc.dram_tensor("bfcr", [64, 1], f32, kind="ExternalInput")
    out_t = nc.dram_tensor("out", [64, 1], f32, kind="ExternalOutput")

    RG = [list(range(N_CORES))]

    with tile.TileContext(nc) as tc:
        with (
            tc.tile_pool(name="dram", bufs=1, space="DRAM") as dram,
            tc.tile_pool(name="const", bufs=1) as const,
            tc.tile_pool(name="cmat", bufs=2) as cpool,
            tc.tile_pool(name="gat", bufs=2) as gpool,
            tc.tile_pool(name="epi", bufs=2) as epool,
            tc.tile_pool(name="sps", bufs=1, space="PSUM") as spool,
        ):
            # DRAM buffers
            T2 = dram.tile([NODES_PAD, TD], bfdt, addr_space="Shared")
            T3 = dram.tile([NODES_PAD, TD], bfdt, addr_space="Shared")
            ag2 = dram.tile([NPC, TD], bfdt)
            ag3 = dram.tile([NPC, TD], bfdt)
            poolin = dram.tile([64, 65], f32)
            poolred = dram.tile([64, 65], f32, addr_space="Shared")

            # constants / resident streams
            iota_i = const.tile([P, P], i32)
            nc.gpsimd.iota(iota_i[:], pattern=[[1, P]], channel_multiplier=0)
            iota_bf = const.tile([P, P], bfdt)
            nc.vector.tensor_copy(out=iota_bf[:], in_=iota_i[:])
            ident = const.tile([P, P], f32)
            make_identity(nc, ident[:])

            sdstb = const.tile([P, NBLK], bfdt)
            nc.sync.dma_start(out=sdstb[:], in_=dst_t[:])
            swdi = const.tile([P, NBLK], bfdt)
            nc.sync.dma_start(out=swdi[:], in_=wdi_t[:])
            sS = const.tile([P, W_PER_CORE * N_GRAPHS], bfdt)
            nc.sync.dma_start(out=sS[:], in_=S_t[:])
            sW1 = const.tile([F_IN + 1, 64], f32)
            nc.sync.dma_start(out=sW1[:], in_=W1s_t[:])
            sW2 = const.tile([65, 128], f32)
            nc.sync.dma_start(out=sW2[:], in_=W2s_t[:])
            sW3 = const.tile([128, 64], f32)
            nc.sync.dma_start(out=sW3[:], in_=W3_t[:])
            sWfc = const.tile([64, 1], f32)
            nc.sync.dma_start(out=sWfc[:], in_=Wfc_t[:])
            sb3 = const.tile([P, 64], f32)
            nc.sync.dma_start(out=sb3[:], in_=b3_t[:])
            sbfc = const.tile([64, 1], f32)
            nc.sync.dma_start(out=sbfc[:], in_=bfc_t[:])

            Town = const.tile([P, W_PER_CORE * TD], bfdt)
            nc.sync.dma_start(out=Town[:], in_=town_t[:])
            dinv = const.tile([P, W_PER_CORE], f32)
            nc.sync.dma_start(out=dinv[:], in_=dinv_t[:])

            # z accumulator in SBUF (f32)
            z_sb = const.tile([P, W_PER_CORE * 64], f32)

            # --------------------------------------------- shared helpers
            def edge_phase(layer, Ttab, F_rhs, epilogue, post_group=None):
                acc = None
                qsel = 0
                for g, (lo, hi) in enumerate(GROUPS):
                    for (cg, cc, b0, b1) in calls:
                        if cg != g:
                            continue
                        qsel = (qsel + 1) % 4
                        nb = b1 - b0
                        n = nb * P
                        sid = gpool.tile([P, MAX_CALL_BLOCKS * 8], i16,
                                         tag="sid", bufs=3)
                        nc.sync.dma_start(out=sid[:, :nb * 8],
                                          in_=idx_t[:, b0 * 8:b1 * 8])
                        gt = gpool.tile([P, MAX_CALL_BLOCKS, TD], bfdt,
                                        tag="g", bufs=2)
                        c0 = cc * CHUNK
                        c1 = min((cc + 1) * CHUNK, NODES_PAD)
                        nc.gpsimd.dma_gather(
                            out_ap=gt[:, :nb, :],
                            in_ap=Ttab[c0:c1, :],
                            idxs_ap=sid[:, :nb * 8],
                            num_idxs=n, num_idxs_reg=n, elem_size=TD,
                            single_packet=False, queue_num=qsel,
                        )
                        Cb = cpool.tile([P, MAX_CALL_BLOCKS, P], bfdt,
                                        tag="C", bufs=2)
                        nc.vector.tensor_tensor(
                            out=Cb[:, :nb, :],
                            in0=iota_bf[:].unsqueeze(1)
                                .to_broadcast([P, nb, P]),
                            in1=sdstb[:, b0:b1].unsqueeze(2)
                                .to_broadcast([P, nb, P]),
                            op=OP.is_equal,
                        )
                        gts = gpool.tile([P, MAX_CALL_BLOCKS, 64], bfdt,
                                         tag="gs", bufs=2)
                        nc.vector.tensor_tensor(
                            out=gts[:, :nb, 0:F_rhs],
                            in0=gt[:, :nb, 0:F_rhs],
                            in1=swdi[:, b0:b1].unsqueeze(2)
                                .to_broadcast([P, nb, F_rhs]),
                            op=OP.mult,
                        )
                        for j in range(nb):
                            b = b0 + j
                            c, wloc, _, st, sp = blocks[b]
                            if st:
                                acc = spool.tile([P, 64], f32, tag="acc",
                                                 bufs=4, name="zacc")
                            nc.tensor.matmul(
                                out=acc[:, 0:F_rhs],
                                lhsT=Cb[:, j, :], rhs=gts[:, j, 0:F_rhs],
                                start=st, stop=sp, skip_group_check=True,
                            )
                            if sp:
                                zs = z_sb[:, wloc * 64:wloc * 64 + F_rhs]
                                if c == 0:
                                    nc.vector.tensor_copy(
                                        out=zs, in_=acc[:, 0:F_rhs])
                                else:
                                    nc.vector.tensor_tensor(
                                        out=zs, in0=zs, in1=acc[:, 0:F_rhs],
                                        op=OP.add)
                    for wloc in range(lo, hi):
                        epilogue(wloc,
                                 z_sb[:, wloc * 64:wloc * 64 + F_rhs])
                    if post_group is not None:
                        post_group(lo, hi)

            # --------------------------------------------- layer 1
            def epi1(w, zsl):
                # self loop: e1 = z + dinv*Town  (z already has w*dinv_dst)
                selfp = epool.tile([P, F_IN], f32, tag="selfp")
                nc.scalar.activation(
                    out=selfp[:], in_=Town[:, w * TD:w * TD + F_IN],
                    func=AF.Copy, scale=dinv[:, w:w + 1])
                e1 = epool.tile([P, F_IN + 1], f32, tag="e1")
                nc.vector.tensor_tensor(out=e1[:, 0:F_IN], in0=zsl,
                                        in1=selfp[:], op=OP.add)
                nc.vector.memset(e1[:, F_IN:F_IN + 1], 1.0)
                tp = spool.tile([P, P], f32, tag="sc1")
                nc.tensor.transpose(out=tp[:F_IN + 1, :], in_=e1[:],
                                    identity=ident[:])
                zT = epool.tile([F_IN + 1, P], f32, tag="zT1")
                nc.vector.tensor_copy(out=zT[:], in_=tp[:F_IN + 1, :])
                hp = spool.tile([P, P], f32, tag="sc2")
                nc.tensor.matmul(out=hp[:, 0:64], lhsT=zT[:], rhs=sW1[:],
                                 start=True, stop=True, skip_group_check=True)
                # Town <- relu(hp) * dinv   (bf16)
                nc.scalar.activation(
                    out=Town[:, w * TD:w * TD + 64], in_=hp[:, 0:64],
                    func=AF.Relu, scale=dinv[:, w:w + 1],
                )
                nc.sync.dma_start(out=ag2[w * P:(w + 1) * P, :],
                                  in_=Town[:, w * TD:(w + 1) * TD])

            edge_phase(1, T1_t, F_IN, epi1)
            nc.gpsimd.collective_compute(
                "AllGather", OP.bypass, replica_groups=RG,
                ins=[ag2.opt()], outs=[T2.opt()],
            )

            # --------------------------------------------- layer 2
            def epi2(w, zsl):
                selfp = epool.tile([P, 64], f32, tag="selfp2")
                nc.scalar.activation(
                    out=selfp[:], in_=Town[:, w * TD:w * TD + 64],
                    func=AF.Copy, scale=dinv[:, w:w + 1])
                e1 = epool.tile([P, 65], f32, tag="e1f2")
                nc.vector.tensor_tensor(out=e1[:, 0:64], in0=zsl,
                                        in1=selfp[:], op=OP.add)
                nc.vector.memset(e1[:, 64:65], 1.0)
                tp = spool.tile([P, P], f32, tag="sc1")
                nc.tensor.transpose(out=tp[:65, :], in_=e1[:],
                                    identity=ident[:])
                zT = epool.tile([65, P], f32, tag="zT2")
                nc.vector.tensor_copy(out=zT[:], in_=tp[:65, :])
                hp = spool.tile([P, P], f32, tag="sc2")
                nc.tensor.matmul(out=hp[:], lhsT=zT[:], rhs=sW2[:],
                                 start=True, stop=True, skip_group_check=True)
                h2 = epool.tile([P, P], f32, tag="h2r")
                nc.scalar.activation(out=h2[:], in_=hp[:], func=AF.Relu)
                tp2 = spool.tile([P, P], f32, tag="sc1")
                nc.tensor.transpose(out=tp2[:], in_=h2[:], identity=ident[:])
                h2T = epool.tile([P, P], f32, tag="h2T")
                nc.vector.tensor_copy(out=h2T[:], in_=tp2[:])
                mp = spool.tile([P, P], f32, tag="sc2")
                nc.tensor.matmul(out=mp[:, 0:64], lhsT=h2T[:], rhs=sW3[:],
                                 start=True, stop=True, skip_group_check=True)
                nc.scalar.activation(
                    out=Town[:, w * TD:w * TD + 64], in_=mp[:, 0:64],
                    func=AF.Copy, scale=dinv[:, w:w + 1],
                )
                nc.sync.dma_start(out=ag3[w * P:(w + 1) * P, :],
                                  in_=Town[:, w * TD:(w + 1) * TD])

            edge_phase(2, T2, 64, epi2)
            nc.gpsimd.collective_compute(
                "AllGather", OP.bypass, replica_groups=RG,
                ins=[ag3.opt()], outs=[T3.opt()],
            )

            # --------------------------------------------- layer 3 + pool
            pool_ps = spool.tile([P, 512], f32, tag="sc3")

            def epi3(w, zsl):
                selfp = epool.tile([P, 64], f32, tag="selfp2")
                nc.scalar.activation(
                    out=selfp[:], in_=Town[:, w * TD:w * TD + 64],
                    func=AF.Copy, scale=dinv[:, w:w + 1])
                e1 = epool.tile([P, 64], f32, tag="e1f")
                nc.vector.tensor_tensor(out=e1[:], in0=zsl, in1=selfp[:],
                                        op=OP.add)
                e2 = epool.tile([P, 64], f32, tag="e2f")
                nc.vector.tensor_tensor(out=e2[:], in0=e1[:], in1=sb3[:],
                                        op=OP.add)
                h3e = epool.tile([P, 65], bfdt, tag="h3e")
                nc.scalar.activation(out=h3e[:, 0:64], in_=e2[:],
                                     func=AF.Relu)
                nc.vector.memset(h3e[:, 64:65], 1.0)
                nc.tensor.matmul(
                    out=pool_ps[:64, 0:65],
                    lhsT=sS[:, w * N_GRAPHS:(w + 1) * N_GRAPHS],
                    rhs=h3e[:],
                    start=(w == 0), stop=(w == W_PER_CORE - 1),
                    skip_group_check=True,
                )

            edge_phase(3, T3, 64, epi3)

            poolsb = epool.tile([64, 65], f32, tag="poolsb")
            nc.vector.tensor_copy(out=poolsb[:], in_=pool_ps[:64, 0:65])
            nc.sync.dma_start(out=poolin[:], in_=poolsb[:])
            nc.gpsimd.collective_compute(
                "AllReduce", OP.add, replica_groups=RG,
                ins=[poolin.opt()], outs=[poolred.opt()],
            )
            pr = epool.tile([64, 65], f32, tag="pr")
            nc.sync.dma_start(out=pr[:], in_=poolred[:])
            cntc = epool.tile([64, 1], f32, tag="cntc")
            nc.vector.tensor_scalar(out=cntc[:], in0=pr[:, 64:65],
                                    scalar1=1.0, scalar2=None, op0=OP.max)
            rcnt = epool.tile([64, 1], f32, tag="rcnt")
            nc.vector.reciprocal(out=rcnt[:], in_=cntc[:])
            mean = epool.tile([64, 64], f32, tag="mean")
            nc.vector.tensor_scalar(out=mean[:], in0=pr[:, 0:64],
                                    scalar1=rcnt[:], scalar2=None,
                                    op0=OP.mult)
            tpf = spool.tile([P, P], f32, tag="sc1")
            nc.tensor.transpose(out=tpf[:64, :64], in_=mean[:],
                                identity=ident[:64, :64])
            meanT = epool.tile([64, 64], f32, tag="meanT")
            nc.vector.tensor_copy(out=meanT[:], in_=tpf[:64, :64])
            op_ps = spool.tile([P, P], f32, tag="sc2")
            nc.tensor.matmul(out=op_ps[:64, 0:1], lhsT=meanT[:], rhs=sWfc[:],
                             start=True, stop=True, skip_group_check=True)
            ob = epool.tile([64, 1], f32, tag="ob")
            nc.vector.tensor_tensor(out=ob[:], in0=op_ps[:64, 0:1],
                                    in1=sbfc[:], op=OP.add)
            nc.sync.dma_start(out=out_t[:], in_=ob[:])

    nc.finalize()
    return nc


# ------------------------------------------------------------------ runner
def _install_ntff_shim():
    try:
        import antenv
        if hasattr(antenv, "axon_hooks"):
            return
        mod = types.ModuleType("antenv.axon_hooks")
        mod._hook = None
        mod.set_axon_ntff_profile_hook = lambda h: setattr(mod, "_hook", h)
        mod.get_axon_ntff_profile_hook = lambda: mod._hook
        sys.modules["antenv.axon_hooks"] = mod
        antenv.axon_hooks = mod
        from trn_agent_boot.trn_boot import _ntff_profile_via_ctypes
        mod._hook = _ntff_profile_via_ctypes("/opt/axon/libaxon_pjrt.so")
    except Exception:
        pass


def kernel(x, edge_index, edge_weight, batch, W1, b1, W2, b2, W3, b3,
           Wfc, bfc):
    global LAST_EXEC_TIME_NS, LAST_TRACE, LAST_RESULT

    x = np.asarray(x, dtype=np.float32)
    ei = np.asarray(edge_index)
    src = ei[0].astype(np.int64)
    dst = ei[1].astype(np.int64)
    w = np.asarray(edge_weight, dtype=np.float32)
    batch = np.asarray(batch)

    deg = np.bincount(dst, weights=w.astype(np.float64),
                      minlength=N_NODES) + 1.0
    dinv_n = (1.0 / np.sqrt(deg)).astype(np.float32)
    meta, idxw, dstrel_st, wdi_st, pos = _prep_edges(src, dst, w, dinv_n)
    town_r, Sbf, dinvf, T1bf = _prep_nodes(x, batch, pos, dinv_n)

    W1 = np.asarray(W1, np.float32)
    W2 = np.asarray(W2, np.float32)
    W3 = np.asarray(W3, np.float32)
    Wfc = np.asarray(Wfc, np.float32).reshape(64, 1)
    W1s = np.vstack([W1, np.asarray(b1, np.float32).reshape(1, 64)])
    W2s = np.vstack([W2, np.asarray(b2, np.float32).reshape(1, 128)])
    b3r = np.tile(np.asarray(b3, np.float32).reshape(1, 64), (P, 1))
    bfcr = np.tile(np.asarray(bfc, np.float32).reshape(1, 1), (64, 1))

    nc = _build_nc(meta)

    in_maps = []
    for k in range(N_CORES):
        in_maps.append({
            "town": town_r[k], "idxw": idxw[k], "dstrel": dstrel_st[k],
            "wdi": wdi_st[k], "Sbf": Sbf[k], "T1bf": T1bf,
            "dinvr": dinvf[k],
            "W1s": W1s, "W2s": W2s, "W3": W3, "Wfc": Wfc,
            "b3r": b3r, "bfcr": bfcr,
        })

    trace = os.environ.get("BASS_GNN_TRACE", "") == "1"
    if trace:
        _install_ntff_shim()
        from concourse import bass_utils as _bu
        _bu.upload_artifacts = lambda tmpdir: tmpdir

    from concourse.bass_utils import run_bass_kernel_spmd
    res = run_bass_kernel_spmd(
        nc, in_maps, core_ids=list(range(N_CORES)), trace=trace,
    )
    LAST_RESULT = res
    if trace:
        LAST_EXEC_TIME_NS = res.exec_time_ns
        LAST_TRACE = (res.instructions_and_trace[1]
                      if res.instructions_and_trace else None)
    return np.asarray(res.results[0]["out"], dtype=np.float32)
